# revision 51
# baseline (speedup 1.0000x reference)
"""Bayesian linear layer on 8 TRN2 NeuronCores.

out = x @ (mu + softplus(rho) * eps) + (bias_mu + softplus(bias_rho) * bias_eps)
x: [8192, 4096] f32, mu/rho/eps: [4096, 4096] f32, out: [8192, 4096] f32.

Sharding: batch 2-way x out_features 4-way (8 cores); all math runs on
device in bf16 with fp32 PSUM accumulation (rel err ~3e-3).

Per core:
- x shard is host-transposed to [in, batch], cast bf16, and tiled to
  [MBT, SBT, KP, 128, 2, 512] so each dma_start moves one contiguous
  256 KiB block with 2 KiB per-partition runs (DMA packet = one run;
  small runs are what killed bandwidth in early versions).
- mu/eps (and rho on the general path) stream as bf16 k-pair
  [128, 2, 1024] tiles, interleaved with the unit-0 x tiles on the sync
  HWDGE ring in exact consumption order, so PE starts consuming w k-pairs
  while later ones still stream. (The scalar HWDGE ring crashes the
  device in this runtime; gpsimd SWDGE carries output writes + early bias
  loads instead.)
- w = mu + softplus(rho)*eps materializes once in SBUF (64 KiB/partition)
  and is reused by all 8 batch units. softplus = Ln(Exp(x)+1) on ACT
  (no Softplus LUT on TRN2); when rho is a constant tensor (it is for
  this problem's init: rho = -3), softplus(rho) folds to a host scalar,
  rho is never transferred, and the whole variant is ACT-free.
- Matmul: lhsT = x tile [128, 128] stationary, rhs = w tile [128, 512]
  moving, k-outer over 8 PSUM banks per 512-batch unit. Unit 0 spreads
  each w k-pair across all banks (paces with the weight ramp); later
  units run group-major so banks release progressively; the final unit
  uses half-tile epilogues on alternating DMA rings to shorten the tail.
- Bias broadcasts across partitions via a ones-matmul once, then rides
  the PSUM->SBUF epilogue add for free.

Measured: ~468 us per core (max core ~474 us), vs ~462 us structural
floor (PE busy ~445 us at the bf16 N=512 issue rate + fixed startup and
drain-barrier overhead).
"""

import ml_dtypes
import numpy as np

import concourse.bacc as bacc
import concourse.bass as bass
import concourse.mybir as mybir
import concourse.tile as tile
from concourse.bass_utils import run_bass_kernel_spmd

F32 = mybir.dt.float32
BF16 = mybir.dt.bfloat16
NP_BF16 = ml_dtypes.bfloat16

IN_F = 4096          # contraction dim (full)
B_CORE = 4096        # batch rows per core (8192 / 2)
O_CORE = 1024        # out features per core (4096 / 4)
P = 128              # SBUF partitions
KT = IN_F // P       # 32 contraction tiles
KP = KT // 2         # 16 k-pairs (DMA granularity)
NB = 1024            # batch block
MBT = B_CORE // NB   # 4 batch blocks per core
SBT = NB // 512      # 2 column groups of 512 within a block
M_SUB = 4            # 4 m-subtiles of 128 within a 512 group
N_MM = 512           # matmul free dim / psum bank width (fp32)
N_SUB = O_CORE // N_MM  # 2 out tiles per core

_CACHE = {}
LAST_RESULT = None


def build_nc(sp_const=None, bsp_const=None):
    """sp_const / bsp_const: softplus(rho) / softplus(bias_rho) as python
    floats when those tensors are constant (skips the rho stream and the
    softplus LUT chain); None -> general path for that tensor."""
    key = ("nc", sp_const, bsp_const)
    if key in _CACHE:
        return _CACHE[key]

    nc = bacc.Bacc("TRN2", target_bir_lowering=False, debug=False)

    xt = nc.dram_tensor(
        "xt", [MBT, SBT, KP, P, 2, 512], BF16, kind="ExternalInput"
    ).ap()
    mu = nc.dram_tensor("mu", [IN_F, O_CORE], BF16, kind="ExternalInput").ap()
    rho = (
        None
        if sp_const is not None
        else nc.dram_tensor("rho", [IN_F, O_CORE], BF16, kind="ExternalInput").ap()
    )
    eps = nc.dram_tensor("eps", [IN_F, O_CORE], BF16, kind="ExternalInput").ap()
    bmu = nc.dram_tensor("bmu", [1, O_CORE], F32, kind="ExternalInput").ap()
    brho = (
        None
        if bsp_const is not None
        else nc.dram_tensor("brho", [1, O_CORE], F32, kind="ExternalInput").ap()
    )
    beps = nc.dram_tensor("beps", [1, O_CORE], F32, kind="ExternalInput").ap()
    out = nc.dram_tensor("out", [B_CORE, O_CORE], F32, kind="ExternalOutput").ap()

    # general path carries extra rho/f32-sp stage tags -> shallower pools
    stage_bufs, outp_bufs = (3, 6) if sp_const is not None else (2, 4)
    with tile.TileContext(nc) as tc:
        with (
            tc.tile_pool(name="wpool", bufs=1) as wpool,
            tc.tile_pool(name="stage", bufs=stage_bufs) as stage,
            tc.tile_pool(name="biasp", bufs=1) as biasp,
            tc.tile_pool(name="xb", bufs=1) as xbp,
            tc.tile_pool(name="outp", bufs=outp_bufs) as outp,
            tc.tile_pool(name="psum", bufs=1, space=bass.MemorySpace.PSUM) as psp,
        ):
            # ---- bias: b = bmu + softplus(brho) * beps, broadcast to 128
            # partitions. Staging borrows the epilogue outp slots (same size,
            # strictly earlier lifetime); chain uses only 2 concurrent slots.
            beps_t = outp.tile([1, O_CORE], F32, tag="o", name="beps_t")
            nc.gpsimd.dma_start(beps_t[:], beps[:])
            bmu_t = outp.tile([1, O_CORE], F32, tag="o", name="bmu_t")
            nc.gpsimd.dma_start(bmu_t[:], bmu[:])
            if bsp_const is not None:
                nc.vector.tensor_scalar_mul(beps_t[:], beps_t[:], float(bsp_const))
            else:
                # softplus(x) = ln(exp(x) + 1) — no Softplus LUT on TRN2
                brho_t = outp.tile([1, O_CORE], F32, tag="o", name="brho_t")
                nc.sync.dma_start(brho_t[:], brho[:])
                nc.scalar.activation(
                    brho_t[:], brho_t[:], mybir.ActivationFunctionType.Exp
                )
                nc.scalar.activation(
                    brho_t[:], brho_t[:], mybir.ActivationFunctionType.Ln, bias=1.0
                )
                nc.vector.tensor_mul(beps_t[:], beps_t[:], brho_t[:])
            bias_row = biasp.tile([1, O_CORE], F32, tag="bias_row")
            nc.vector.tensor_add(bias_row[:], bmu_t[:], beps_t[:])
            # broadcast to all partitions on gpsimd — keeps PE's queue head
            # free for the first real matmuls
            bias_bc = wpool.tile([P, O_CORE], F32, tag="bias_bc")
            nc.gpsimd.partition_broadcast(bias_bc[:], bias_row[:])

            # ---- weights: w = mu + softplus(rho) * eps, bf16, resident.
            # Loaded as k-pairs [128, 2, O_CORE] (contiguous 512 KiB per DMA)
            # on the gpsimd SWDGE queue so they run in parallel with the
            # sync-ring x stream; the mb=0 x loads are interleaved in issue
            # order so PE can start consuming k-pairs as both arrive.
            w_bf = []
            xb0_tiles = []
            for kp in range(KP):
                rsl = slice(kp * 2 * P, (kp + 1) * 2 * P)
                mu_t = stage.tile([P, 2, O_CORE], BF16, tag="mu")
                eps_t = stage.tile([P, 2, O_CORE], BF16, tag="eps")
                if sp_const is not None and kp == 0:
                    # first k-pair at half granularity: halves the serial
                    # DMA->DVE chain ahead of the very first matmul (Tile
                    # tracks subtile deps, so mm(k=0) only waits on half 0)
                    sp_t = stage.tile([P, 2, O_CORE], BF16, tag="spb")
                    w_t = wpool.tile([P, 2, O_CORE], BF16, tag=f"w{kp}")
                    for kk in range(2):
                        hsl = slice(kp * 2 * P + kk * P, kp * 2 * P + (kk + 1) * P)
                        nc.sync.dma_start(eps_t[:, kk], eps[hsl, :])
                        nc.sync.dma_start(mu_t[:, kk], mu[hsl, :])
                        nc.vector.tensor_scalar_mul(
                            sp_t[:, kk], eps_t[:, kk], float(sp_const)
                        )
                        nc.vector.tensor_add(
                            w_t[:, kk], mu_t[:, kk], sp_t[:, kk]
                        )
                        if kk == 0:
                            xb_t = xbp.tile(
                                [P, 2, 512], BF16, tag="xb0_0", name="xb0_0"
                            )
                            nc.sync.dma_start(xb_t[:], xt[0, 0, 0])
                            xb0_tiles.append(xb_t)
                    w_bf.append(w_t)
                    continue
                nc.sync.dma_start(
                    eps_t[:], eps[rsl, :].rearrange("(kk p) j -> p kk j", p=P)
                )
                nc.sync.dma_start(
                    mu_t[:], mu[rsl, :].rearrange("(kk p) j -> p kk j", p=P)
                )
                if sp_const is not None:
                    # const path: no ACT at all (no LUT table loads); bf16
                    # scaled copy + add on DVE.
                    sp_t = stage.tile([P, 2, O_CORE], BF16, tag="spb")
                    nc.vector.tensor_scalar_mul(
                        sp_t[:], eps_t[:], float(sp_const)
                    )
                else:
                    sp_t = stage.tile([P, 2, O_CORE], F32, tag="sp")
                    rho_t = stage.tile([P, 2, O_CORE], BF16, tag="rho")
                    nc.sync.dma_start(
                        rho_t[:], rho[rsl, :].rearrange("(kk p) j -> p kk j", p=P)
                    )
                    nc.scalar.activation(
                        sp_t[:], rho_t[:], mybir.ActivationFunctionType.Exp
                    )
                    nc.scalar.activation(
                        sp_t[:], sp_t[:], mybir.ActivationFunctionType.Ln, bias=1.0
                    )
                    nc.vector.tensor_mul(sp_t[:], sp_t[:], eps_t[:])
                w_t = wpool.tile([P, 2, O_CORE], BF16, tag=f"w{kp}")
                nc.vector.tensor_add(w_t[:], mu_t[:], sp_t[:])
                w_bf.append(w_t)

                xb_t = xbp.tile([P, 2, 512], BF16, tag=f"xb{kp}_0", name=f"xb0_{kp}")
                nc.sync.dma_start(xb_t[:], xt[0, 0, kp])
                xb0_tiles.append(xb_t)

            # ---- main loop: one unit per (block, 512-column group);
            # k-outer into 8 psum banks. Final unit runs group-major so its
            # epilogues overlap the last matmuls.
            units = [(mb, sb) for mb in range(MBT) for sb in range(SBT)]
            for u, (mb, sb) in enumerate(units):
                if u == 0:
                    xb_tiles = xb0_tiles
                else:
                    xb_tiles = []
                    for kp in range(KP):
                        xb_t = xbp.tile(
                            [P, 2, 512], BF16,
                            tag=f"xb{kp}_{u % 2}", name=f"xb{u}_{kp}",
                        )
                        nc.sync.dma_start(xb_t[:], xt[mb, sb, kp])
                        xb_tiles.append(xb_t)

                psums = [
                    psp.tile([P, N_MM], F32, tag=f"ps{g}", name=f"ps{g}")
                    for g in range(M_SUB * N_SUB)
                ]

                def mm(k, m, n):
                    kp, kk = divmod(k, 2)
                    nc.tensor.matmul(
                        psums[m * N_SUB + n][:],
                        xb_tiles[kp][:, kk, bass.ts(m, P)],
                        w_bf[kp][:, kk, bass.ts(n, N_MM)],
                        start=(k == 0),
                        stop=(k == KT - 1),
                    )

                def epilogue(m):
                    o_t = outp.tile([P, O_CORE], F32, tag="o", name=f"o{u}_{m}")
                    for n in range(N_SUB):
                        nc.vector.tensor_add(
                            o_t[:, bass.ts(n, N_MM)],
                            psums[m * N_SUB + n][:],
                            bias_bc[:, bass.ts(n, N_MM)],
                        )
                    r0 = mb * NB + sb * 512 + m * P
                    nc.gpsimd.dma_start(out[r0 : r0 + P, :], o_t[:])

                if u == 0:
                    # paced by the weight ramp: spread each w k-pair across
                    # all 8 psum groups so PE consumes w at production rate
                    for k in range(KT):
                        for m in range(M_SUB):
                            for n in range(N_SUB):
                                mm(k, m, n)
                    for m in range(M_SUB):
                        epilogue(m)
                elif u < len(units) - 1:
                    # group-major: psum banks release progressively, so the
                    # next unit never waits on an epilogue burst
                    for m in range(M_SUB):
                        for k in range(KT):
                            for n in range(N_SUB):
                                mm(k, m, n)
                        epilogue(m)
                else:
                    # final unit: half-tile epilogues on alternating DMA
                    # rings so the serial tail after the last matmul is short
                    for m in range(M_SUB):
                        for n in range(N_SUB):
                            for k in range(KT):
                                mm(k, m, n)
                            o_t = outp.tile(
                                [P, N_MM], F32, tag="o", name=f"of{m}_{n}"
                            )
                            nc.vector.tensor_add(
                                o_t[:],
                                psums[m * N_SUB + n][:],
                                bias_bc[:, bass.ts(n, N_MM)],
                            )
                            r0 = mb * NB + sb * 512 + m * P
                            eng = nc.sync if n == 0 else nc.gpsimd
                            eng.dma_start(
                                out[r0 : r0 + P, bass.ts(n, N_MM)], o_t[:]
                            )

    nc.compile()
    _CACHE[key] = nc
    return nc


def kernel(x, mu, rho, bias_mu, bias_rho, epsilon, bias_epsilon):
    global LAST_RESULT
    x = np.asarray(x, dtype=np.float32)
    rho = np.asarray(rho, dtype=np.float32)
    bias_mu = np.asarray(bias_mu, dtype=np.float32).reshape(1, -1)
    bias_rho = np.asarray(bias_rho, dtype=np.float32).reshape(1, -1)
    bias_epsilon = np.asarray(bias_epsilon, dtype=np.float32).reshape(1, -1)
    rho0 = float(rho.flat[0])
    sp_const = (
        float(np.log1p(np.exp(rho0))) if bool(np.all(rho == rho0)) else None
    )
    brho0 = float(bias_rho.flat[0])
    bsp_const = (
        float(np.log1p(np.exp(brho0)))
        if bool(np.all(bias_rho == brho0))
        else None
    )

    B_SH, O_SH = 2, 4
    # x shard -> [in, batch] bf16, tiled [MBT, SBT, KP, P, 2, 512] so each
    # (mb, sb, kp) block is one contiguous 256 KiB region with 2 KiB
    # per-partition runs.
    xt_s = []
    for b in range(B_SH):
        xT = x[b * B_CORE : (b + 1) * B_CORE].T.astype(NP_BF16)  # [IN_F, B_CORE]
        xt_s.append(
            np.ascontiguousarray(
                xT.reshape(KP, 2, P, MBT, SBT, 512).transpose(3, 4, 0, 2, 1, 5)
            )
        )

    def wslices(t):
        t = np.asarray(t, dtype=np.float32)
        return [
            np.ascontiguousarray(t[:, o * O_CORE : (o + 1) * O_CORE].astype(NP_BF16))
            for o in range(O_SH)
        ]

    mu_s, eps_s = wslices(mu), wslices(epsilon)
    rho_s = None if sp_const is not None else wslices(rho)
    bmu_s = [np.ascontiguousarray(bias_mu[:, o * O_CORE : (o + 1) * O_CORE]) for o in range(O_SH)]
    brho_s = [np.ascontiguousarray(bias_rho[:, o * O_CORE : (o + 1) * O_CORE]) for o in range(O_SH)]
    beps_s = [np.ascontiguousarray(bias_epsilon[:, o * O_CORE : (o + 1) * O_CORE]) for o in range(O_SH)]

    in_maps = []
    for c in range(8):
        b, o = divmod(c, O_SH)
        m = {
            "xt": xt_s[b],
            "mu": mu_s[o],
            "eps": eps_s[o],
            "bmu": bmu_s[o],
            "beps": beps_s[o],
        }
        if rho_s is not None:
            m["rho"] = rho_s[o]
        if bsp_const is None:
            m["brho"] = brho_s[o]
        in_maps.append(m)

    nc = build_nc(sp_const, bsp_const)
    res = run_bass_kernel_spmd(nc, in_maps, list(range(8)))
    LAST_RESULT = res

    out = np.empty((B_SH * B_CORE, O_SH * O_CORE), dtype=np.float32)
    for c in range(8):
        b, o = divmod(c, O_SH)
        out[b * B_CORE : (b + 1) * B_CORE, o * O_CORE : (o + 1) * O_CORE] = (
            res.results[c]["out"]
        )
    return out


# revision 52
# speedup vs baseline: 1.0131x; 1.0131x over previous
"""Bayesian linear layer on 8 TRN2 NeuronCores.

out = x @ (mu + softplus(rho) * eps) + (bias_mu + softplus(bias_rho) * bias_eps)
x: [8192, 4096] f32, mu/rho/eps: [4096, 4096] f32, out: [8192, 4096] f32.

Sharding: batch 2-way x out_features 4-way (8 cores); all math runs on
device in bf16 with fp32 PSUM accumulation (rel err ~3e-3).

Per core:
- x shard is host-transposed to [in, batch], cast bf16, and tiled to
  [MBT, SBT, KP, 128, 2, 512] so each dma_start moves one contiguous
  256 KiB block with 2 KiB per-partition runs (DMA packet = one run;
  small runs are what killed bandwidth in early versions).
- mu/eps (and rho on the general path) stream as bf16 k-pair
  [128, 2, 1024] tiles, interleaved with the unit-0 x tiles on the sync
  HWDGE ring in exact consumption order, so PE starts consuming w k-pairs
  while later ones still stream. (The scalar HWDGE ring crashes the
  device in this runtime; gpsimd SWDGE carries output writes + early bias
  loads instead.)
- w = mu + softplus(rho)*eps materializes once in SBUF (64 KiB/partition)
  and is reused by all 8 batch units. softplus = Ln(Exp(x)+1) on ACT
  (no Softplus LUT on TRN2); when rho is a constant tensor (it is for
  this problem's init: rho = -3), softplus(rho) folds to a host scalar,
  rho is never transferred, and the whole variant is ACT-free.
- Matmul: lhsT = x tile [128, 128] stationary, rhs = w tile [128, 512]
  moving, k-outer over 8 PSUM banks per 512-batch unit. Unit 0 spreads
  each w k-pair across all banks (paces with the weight ramp); later
  units run group-major so banks release progressively; the final unit
  uses half-tile epilogues on alternating DMA rings to shorten the tail.
- Bias broadcasts across partitions once via gpsimd partition_broadcast
  (keeps PE's queue head free), then rides the PSUM->SBUF epilogue add
  for free. The first w k-pair builds at half granularity so the very
  first matmul's DMA->DVE dependency chain is short.

Measured: ~467-475 us per core, vs ~462 us structural floor (PE busy
~446 us at the bf16 N=512 issue rate, plus fixed NEFF preamble, HWDGE
ring spin-up, and drain-barrier overhead).
"""

import ml_dtypes
import numpy as np

import concourse.bacc as bacc
import concourse.bass as bass
import concourse.mybir as mybir
import concourse.tile as tile
from concourse.bass_utils import run_bass_kernel_spmd

F32 = mybir.dt.float32
BF16 = mybir.dt.bfloat16
NP_BF16 = ml_dtypes.bfloat16

IN_F = 4096          # contraction dim (full)
B_CORE = 4096        # batch rows per core (8192 / 2)
O_CORE = 1024        # out features per core (4096 / 4)
P = 128              # SBUF partitions
KT = IN_F // P       # 32 contraction tiles
KP = KT // 2         # 16 k-pairs (DMA granularity)
NB = 1024            # batch block
MBT = B_CORE // NB   # 4 batch blocks per core
SBT = NB // 512      # 2 column groups of 512 within a block
M_SUB = 4            # 4 m-subtiles of 128 within a 512 group
N_MM = 512           # matmul free dim / psum bank width (fp32)
N_SUB = O_CORE // N_MM  # 2 out tiles per core

_CACHE = {}
LAST_RESULT = None


def build_nc(sp_const=None, bsp_const=None):
    """sp_const / bsp_const: softplus(rho) / softplus(bias_rho) as python
    floats when those tensors are constant (skips the rho stream and the
    softplus LUT chain); None -> general path for that tensor."""
    key = ("nc", sp_const, bsp_const)
    if key in _CACHE:
        return _CACHE[key]

    nc = bacc.Bacc("TRN2", target_bir_lowering=False, debug=False)

    xt = nc.dram_tensor(
        "xt", [MBT, SBT, KP, P, 2, 512], BF16, kind="ExternalInput"
    ).ap()
    mu = nc.dram_tensor("mu", [IN_F, O_CORE], BF16, kind="ExternalInput").ap()
    rho = (
        None
        if sp_const is not None
        else nc.dram_tensor("rho", [IN_F, O_CORE], BF16, kind="ExternalInput").ap()
    )
    eps = nc.dram_tensor("eps", [IN_F, O_CORE], BF16, kind="ExternalInput").ap()
    bmu = nc.dram_tensor("bmu", [1, O_CORE], F32, kind="ExternalInput").ap()
    brho = (
        None
        if bsp_const is not None
        else nc.dram_tensor("brho", [1, O_CORE], F32, kind="ExternalInput").ap()
    )
    beps = nc.dram_tensor("beps", [1, O_CORE], F32, kind="ExternalInput").ap()
    out = nc.dram_tensor("out", [B_CORE, O_CORE], F32, kind="ExternalOutput").ap()

    # general path carries extra rho/f32-sp stage tags -> shallower pools
    stage_bufs, outp_bufs = (3, 6) if sp_const is not None else (2, 4)
    with tile.TileContext(nc) as tc:
        with (
            tc.tile_pool(name="wpool", bufs=1) as wpool,
            tc.tile_pool(name="stage", bufs=stage_bufs) as stage,
            tc.tile_pool(name="biasp", bufs=1) as biasp,
            tc.tile_pool(name="xb", bufs=1) as xbp,
            tc.tile_pool(name="outp", bufs=outp_bufs) as outp,
            tc.tile_pool(name="psum", bufs=1, space=bass.MemorySpace.PSUM) as psp,
        ):
            # ---- bias: b = bmu + softplus(brho) * beps, broadcast to 128
            # partitions. Staging borrows the epilogue outp slots (same size,
            # strictly earlier lifetime); chain uses only 2 concurrent slots.
            beps_t = outp.tile([1, O_CORE], F32, tag="o", name="beps_t")
            nc.gpsimd.dma_start(beps_t[:], beps[:])
            bmu_t = outp.tile([1, O_CORE], F32, tag="o", name="bmu_t")
            nc.gpsimd.dma_start(bmu_t[:], bmu[:])
            if bsp_const is not None:
                nc.vector.tensor_scalar_mul(beps_t[:], beps_t[:], float(bsp_const))
            else:
                # softplus(x) = ln(exp(x) + 1) — no Softplus LUT on TRN2
                brho_t = outp.tile([1, O_CORE], F32, tag="o", name="brho_t")
                nc.sync.dma_start(brho_t[:], brho[:])
                nc.scalar.activation(
                    brho_t[:], brho_t[:], mybir.ActivationFunctionType.Exp
                )
                nc.scalar.activation(
                    brho_t[:], brho_t[:], mybir.ActivationFunctionType.Ln, bias=1.0
                )
                nc.vector.tensor_mul(beps_t[:], beps_t[:], brho_t[:])
            bias_row = biasp.tile([1, O_CORE], F32, tag="bias_row")
            nc.vector.tensor_add(bias_row[:], bmu_t[:], beps_t[:])
            # broadcast to all partitions on gpsimd — keeps PE's queue head
            # free for the first real matmuls
            bias_bc = wpool.tile([P, O_CORE], F32, tag="bias_bc")
            nc.gpsimd.partition_broadcast(bias_bc[:], bias_row[:])

            # ---- weights: w = mu + softplus(rho) * eps, bf16, resident.
            # Loaded as k-pairs [128, 2, O_CORE] (contiguous 512 KiB per DMA)
            # on the gpsimd SWDGE queue so they run in parallel with the
            # sync-ring x stream; the mb=0 x loads are interleaved in issue
            # order so PE can start consuming k-pairs as both arrive.
            w_bf = []
            xb0_tiles = []
            for kp in range(KP):
                rsl = slice(kp * 2 * P, (kp + 1) * 2 * P)
                mu_t = stage.tile([P, 2, O_CORE], BF16, tag="mu")
                eps_t = stage.tile([P, 2, O_CORE], BF16, tag="eps")
                if sp_const is not None and kp == 0:
                    # first k-pair at half granularity: halves the serial
                    # DMA->DVE chain ahead of the very first matmul (Tile
                    # tracks subtile deps, so mm(k=0) only waits on half 0)
                    sp_t = stage.tile([P, 2, O_CORE], BF16, tag="spb")
                    w_t = wpool.tile([P, 2, O_CORE], BF16, tag=f"w{kp}")
                    for kk in range(2):
                        hsl = slice(kp * 2 * P + kk * P, kp * 2 * P + (kk + 1) * P)
                        nc.sync.dma_start(eps_t[:, kk], eps[hsl, :])
                        nc.sync.dma_start(mu_t[:, kk], mu[hsl, :])
                        nc.vector.tensor_scalar_mul(
                            sp_t[:, kk], eps_t[:, kk], float(sp_const)
                        )
                        nc.vector.tensor_add(
                            w_t[:, kk], mu_t[:, kk], sp_t[:, kk]
                        )
                        if kk == 0:
                            xb_t = xbp.tile(
                                [P, 2, 512], BF16, tag="xb0_0", name="xb0_0"
                            )
                            nc.sync.dma_start(xb_t[:], xt[0, 0, 0])
                            xb0_tiles.append(xb_t)
                    w_bf.append(w_t)
                    continue
                nc.sync.dma_start(
                    eps_t[:], eps[rsl, :].rearrange("(kk p) j -> p kk j", p=P)
                )
                nc.sync.dma_start(
                    mu_t[:], mu[rsl, :].rearrange("(kk p) j -> p kk j", p=P)
                )
                if sp_const is not None:
                    # const path: no ACT at all (no LUT table loads); bf16
                    # scaled copy + add on DVE.
                    sp_t = stage.tile([P, 2, O_CORE], BF16, tag="spb")
                    nc.vector.tensor_scalar_mul(
                        sp_t[:], eps_t[:], float(sp_const)
                    )
                else:
                    sp_t = stage.tile([P, 2, O_CORE], F32, tag="sp")
                    rho_t = stage.tile([P, 2, O_CORE], BF16, tag="rho")
                    nc.sync.dma_start(
                        rho_t[:], rho[rsl, :].rearrange("(kk p) j -> p kk j", p=P)
                    )
                    nc.scalar.activation(
                        sp_t[:], rho_t[:], mybir.ActivationFunctionType.Exp
                    )
                    nc.scalar.activation(
                        sp_t[:], sp_t[:], mybir.ActivationFunctionType.Ln, bias=1.0
                    )
                    nc.vector.tensor_mul(sp_t[:], sp_t[:], eps_t[:])
                w_t = wpool.tile([P, 2, O_CORE], BF16, tag=f"w{kp}")
                nc.vector.tensor_add(w_t[:], mu_t[:], sp_t[:])
                w_bf.append(w_t)

                xb_t = xbp.tile([P, 2, 512], BF16, tag=f"xb{kp}_0", name=f"xb0_{kp}")
                nc.sync.dma_start(xb_t[:], xt[0, 0, kp])
                xb0_tiles.append(xb_t)

            # ---- main loop: one unit per (block, 512-column group);
            # k-outer into 8 psum banks. Final unit runs group-major so its
            # epilogues overlap the last matmuls.
            units = [(mb, sb) for mb in range(MBT) for sb in range(SBT)]
            for u, (mb, sb) in enumerate(units):
                if u == 0:
                    xb_tiles = xb0_tiles
                else:
                    xb_tiles = []
                    for kp in range(KP):
                        xb_t = xbp.tile(
                            [P, 2, 512], BF16,
                            tag=f"xb{kp}_{u % 2}", name=f"xb{u}_{kp}",
                        )
                        nc.sync.dma_start(xb_t[:], xt[mb, sb, kp])
                        xb_tiles.append(xb_t)

                psums = [
                    psp.tile([P, N_MM], F32, tag=f"ps{g}", name=f"ps{g}")
                    for g in range(M_SUB * N_SUB)
                ]

                def mm(k, m, n):
                    kp, kk = divmod(k, 2)
                    nc.tensor.matmul(
                        psums[m * N_SUB + n][:],
                        xb_tiles[kp][:, kk, bass.ts(m, P)],
                        w_bf[kp][:, kk, bass.ts(n, N_MM)],
                        start=(k == 0),
                        stop=(k == KT - 1),
                    )

                def epilogue(m):
                    o_t = outp.tile([P, O_CORE], F32, tag="o", name=f"o{u}_{m}")
                    for n in range(N_SUB):
                        nc.vector.tensor_add(
                            o_t[:, bass.ts(n, N_MM)],
                            psums[m * N_SUB + n][:],
                            bias_bc[:, bass.ts(n, N_MM)],
                        )
                    r0 = mb * NB + sb * 512 + m * P
                    nc.gpsimd.dma_start(out[r0 : r0 + P, :], o_t[:])

                if u == 0:
                    # paced by the weight ramp: spread each w k-pair across
                    # all 8 psum groups so PE consumes w at production rate
                    for k in range(KT):
                        for m in range(M_SUB):
                            for n in range(N_SUB):
                                mm(k, m, n)
                    for m in range(M_SUB):
                        epilogue(m)
                elif u < len(units) - 1:
                    # group-major: psum banks release progressively, so the
                    # next unit never waits on an epilogue burst
                    for m in range(M_SUB):
                        for k in range(KT):
                            for n in range(N_SUB):
                                mm(k, m, n)
                        epilogue(m)
                else:
                    # final unit: half-tile epilogues on alternating DMA
                    # rings so the serial tail after the last matmul is short
                    for m in range(M_SUB):
                        for n in range(N_SUB):
                            for k in range(KT):
                                mm(k, m, n)
                            o_t = outp.tile(
                                [P, N_MM], F32, tag="o", name=f"of{m}_{n}"
                            )
                            nc.vector.tensor_add(
                                o_t[:],
                                psums[m * N_SUB + n][:],
                                bias_bc[:, bass.ts(n, N_MM)],
                            )
                            r0 = mb * NB + sb * 512 + m * P
                            eng = nc.sync if n == 0 else nc.gpsimd
                            eng.dma_start(
                                out[r0 : r0 + P, bass.ts(n, N_MM)], o_t[:]
                            )

    nc.compile()
    _CACHE[key] = nc
    return nc


def kernel(x, mu, rho, bias_mu, bias_rho, epsilon, bias_epsilon):
    global LAST_RESULT
    x = np.asarray(x, dtype=np.float32)
    rho = np.asarray(rho, dtype=np.float32)
    bias_mu = np.asarray(bias_mu, dtype=np.float32).reshape(1, -1)
    bias_rho = np.asarray(bias_rho, dtype=np.float32).reshape(1, -1)
    bias_epsilon = np.asarray(bias_epsilon, dtype=np.float32).reshape(1, -1)
    rho0 = float(rho.flat[0])
    sp_const = (
        float(np.log1p(np.exp(rho0))) if bool(np.all(rho == rho0)) else None
    )
    brho0 = float(bias_rho.flat[0])
    bsp_const = (
        float(np.log1p(np.exp(brho0)))
        if bool(np.all(bias_rho == brho0))
        else None
    )

    B_SH, O_SH = 2, 4
    # x shard -> [in, batch] bf16, tiled [MBT, SBT, KP, P, 2, 512] so each
    # (mb, sb, kp) block is one contiguous 256 KiB region with 2 KiB
    # per-partition runs.
    xt_s = []
    for b in range(B_SH):
        xT = x[b * B_CORE : (b + 1) * B_CORE].T.astype(NP_BF16)  # [IN_F, B_CORE]
        xt_s.append(
            np.ascontiguousarray(
                xT.reshape(KP, 2, P, MBT, SBT, 512).transpose(3, 4, 0, 2, 1, 5)
            )
        )

    def wslices(t):
        t = np.asarray(t, dtype=np.float32)
        return [
            np.ascontiguousarray(t[:, o * O_CORE : (o + 1) * O_CORE].astype(NP_BF16))
            for o in range(O_SH)
        ]

    mu_s, eps_s = wslices(mu), wslices(epsilon)
    rho_s = None if sp_const is not None else wslices(rho)
    bmu_s = [np.ascontiguousarray(bias_mu[:, o * O_CORE : (o + 1) * O_CORE]) for o in range(O_SH)]
    brho_s = [np.ascontiguousarray(bias_rho[:, o * O_CORE : (o + 1) * O_CORE]) for o in range(O_SH)]
    beps_s = [np.ascontiguousarray(bias_epsilon[:, o * O_CORE : (o + 1) * O_CORE]) for o in range(O_SH)]

    in_maps = []
    for c in range(8):
        b, o = divmod(c, O_SH)
        m = {
            "xt": xt_s[b],
            "mu": mu_s[o],
            "eps": eps_s[o],
            "bmu": bmu_s[o],
            "beps": beps_s[o],
        }
        if rho_s is not None:
            m["rho"] = rho_s[o]
        if bsp_const is None:
            m["brho"] = brho_s[o]
        in_maps.append(m)

    nc = build_nc(sp_const, bsp_const)
    res = run_bass_kernel_spmd(nc, in_maps, list(range(8)))
    LAST_RESULT = res

    out = np.empty((B_SH * B_CORE, O_SH * O_CORE), dtype=np.float32)
    for c in range(8):
        b, o = divmod(c, O_SH)
        out[b * B_CORE : (b + 1) * B_CORE, o * O_CORE : (o + 1) * O_CORE] = (
            res.results[c]["out"]
        )
    return out


# revision 53
# speedup vs baseline: 1.0162x; 1.0031x over previous
"""Bayesian linear layer on 8 TRN2 NeuronCores.

out = x @ (mu + softplus(rho) * eps) + (bias_mu + softplus(bias_rho) * bias_eps)
x: [8192, 4096] f32, mu/rho/eps: [4096, 4096] f32, out: [8192, 4096] f32.

Sharding: batch 2-way x out_features 4-way (8 cores); all math runs on
device in bf16 with fp32 PSUM accumulation (rel err ~3e-3).

Per core:
- x shard is host-transposed to [in, batch], cast bf16, and tiled to
  [MBT, SBT, KP, 128, 2, 512] so each dma_start moves one contiguous
  256 KiB block with 2 KiB per-partition runs (DMA packet = one run;
  small runs are what killed bandwidth in early versions).
- mu/eps (and rho on the general path) stream as bf16 k-pair
  [128, 2, 1024] tiles, interleaved with the unit-0 x tiles on the sync
  HWDGE ring in exact consumption order, so PE starts consuming w k-pairs
  while later ones still stream. (The scalar HWDGE ring crashes the
  device in this runtime; gpsimd SWDGE carries output writes + early bias
  loads instead.)
- w = mu + softplus(rho)*eps materializes once in SBUF (64 KiB/partition)
  and is reused by all 8 batch units. softplus = Ln(Exp(x)+1) on ACT
  (no Softplus LUT on TRN2); when rho is a constant tensor (it is for
  this problem's init: rho = -3), softplus(rho) folds to a host scalar,
  rho is never transferred, and the whole variant is ACT-free.
- Matmul: lhsT = x tile [128, 128] stationary, rhs = w tile [128, 512]
  moving, k-outer over 8 PSUM banks per 512-batch unit. Unit 0 spreads
  each w k-pair across all banks (paces with the weight ramp); later
  units run group-major so banks release progressively; the final unit
  uses half-tile epilogues on alternating DMA rings to shorten the tail.
- Bias broadcasts across partitions once via gpsimd partition_broadcast
  (keeps PE's queue head free), then rides the PSUM->SBUF epilogue add
  for free. The first w k-pair builds at half granularity so the very
  first matmul's DMA->DVE dependency chain is short.

Measured: ~467-475 us per core, vs ~462 us structural floor (PE busy
~446 us at the bf16 N=512 issue rate, plus fixed NEFF preamble, HWDGE
ring spin-up, and drain-barrier overhead).
"""

import ml_dtypes
import numpy as np

import concourse.bacc as bacc
import concourse.bass as bass
import concourse.mybir as mybir
import concourse.tile as tile
from concourse.bass_utils import run_bass_kernel_spmd

F32 = mybir.dt.float32
BF16 = mybir.dt.bfloat16
NP_BF16 = ml_dtypes.bfloat16

IN_F = 4096          # contraction dim (full)
B_CORE = 4096        # batch rows per core (8192 / 2)
O_CORE = 1024        # out features per core (4096 / 4)
P = 128              # SBUF partitions
KT = IN_F // P       # 32 contraction tiles
KP = KT // 2         # 16 k-pairs (DMA granularity)
NB = 1024            # batch block
MBT = B_CORE // NB   # 4 batch blocks per core
SBT = NB // 512      # 2 column groups of 512 within a block
M_SUB = 4            # 4 m-subtiles of 128 within a 512 group
N_MM = 512           # matmul free dim / psum bank width (fp32)
N_SUB = O_CORE // N_MM  # 2 out tiles per core

_CACHE = {}
LAST_RESULT = None


def build_nc(sp_const=None, bsp_const=None):
    """sp_const / bsp_const: softplus(rho) / softplus(bias_rho) as python
    floats when those tensors are constant (skips the rho stream and the
    softplus LUT chain); None -> general path for that tensor."""
    key = ("nc", sp_const, bsp_const)
    if key in _CACHE:
        return _CACHE[key]

    nc = bacc.Bacc("TRN2", target_bir_lowering=False, debug=False)

    xt = nc.dram_tensor(
        "xt", [MBT, SBT, KP, P, 2, 512], BF16, kind="ExternalInput"
    ).ap()
    mu = nc.dram_tensor("mu", [IN_F, O_CORE], BF16, kind="ExternalInput").ap()
    rho = (
        None
        if sp_const is not None
        else nc.dram_tensor("rho", [IN_F, O_CORE], BF16, kind="ExternalInput").ap()
    )
    eps = nc.dram_tensor("eps", [IN_F, O_CORE], BF16, kind="ExternalInput").ap()
    bmu = nc.dram_tensor("bmu", [1, O_CORE], F32, kind="ExternalInput").ap()
    brho = (
        None
        if bsp_const is not None
        else nc.dram_tensor("brho", [1, O_CORE], F32, kind="ExternalInput").ap()
    )
    beps = nc.dram_tensor("beps", [1, O_CORE], F32, kind="ExternalInput").ap()
    out = nc.dram_tensor("out", [B_CORE, O_CORE], F32, kind="ExternalOutput").ap()

    # general path carries extra rho/f32-sp stage tags -> shallower pools
    stage_bufs, outp_bufs = (3, 6) if sp_const is not None else (2, 4)
    with tile.TileContext(nc) as tc:
        with (
            tc.tile_pool(name="wpool", bufs=1) as wpool,
            tc.tile_pool(name="stage", bufs=stage_bufs) as stage,
            tc.tile_pool(name="biasp", bufs=1) as biasp,
            tc.tile_pool(name="xb", bufs=1) as xbp,
            tc.tile_pool(name="outp", bufs=outp_bufs) as outp,
            tc.tile_pool(name="psum", bufs=1, space=bass.MemorySpace.PSUM) as psp,
        ):
            # ---- bias: b = bmu + softplus(brho) * beps, broadcast to 128
            # partitions. Staging borrows the epilogue outp slots (same size,
            # strictly earlier lifetime); chain uses only 2 concurrent slots.
            beps_t = outp.tile([1, O_CORE], F32, tag="o", name="beps_t")
            nc.gpsimd.dma_start(beps_t[:], beps[:])
            bmu_t = outp.tile([1, O_CORE], F32, tag="o", name="bmu_t")
            nc.gpsimd.dma_start(bmu_t[:], bmu[:])
            if bsp_const is not None:
                nc.vector.tensor_scalar_mul(beps_t[:], beps_t[:], float(bsp_const))
            else:
                # softplus(x) = ln(exp(x) + 1) — no Softplus LUT on TRN2
                brho_t = outp.tile([1, O_CORE], F32, tag="o", name="brho_t")
                nc.sync.dma_start(brho_t[:], brho[:])
                nc.scalar.activation(
                    brho_t[:], brho_t[:], mybir.ActivationFunctionType.Exp
                )
                nc.scalar.activation(
                    brho_t[:], brho_t[:], mybir.ActivationFunctionType.Ln, bias=1.0
                )
                nc.vector.tensor_mul(beps_t[:], beps_t[:], brho_t[:])
            bias_row = biasp.tile([1, O_CORE], F32, tag="bias_row")
            nc.vector.tensor_add(bias_row[:], bmu_t[:], beps_t[:])
            # broadcast to all partitions on gpsimd — keeps PE's queue head
            # free for the first real matmuls
            bias_bc = wpool.tile([P, O_CORE], F32, tag="bias_bc")
            nc.gpsimd.partition_broadcast(bias_bc[:], bias_row[:])

            # ---- PE warmup: the HAM clock gate holds PE at 1.2 GHz until
            # ~3.4 us of sustained activity. PE is otherwise idle from ~4 us
            # (preamble done) to ~12 us (first weights landed); burn that
            # window on data-independent matmuls so real work starts at
            # 2.4 GHz.
            wrm_lhs = biasp.tile([1, P], BF16, tag="wrm_lhs")
            wrm_rhs = biasp.tile([1, N_MM], BF16, tag="wrm_rhs")
            nc.vector.memset(wrm_lhs[:], 1.0)
            nc.vector.memset(wrm_rhs[:], 0.0)
            wrm_ps = psp.tile([P, N_MM], F32, tag="ps7", name="warm")
            N_WARM = 20
            for i in range(N_WARM):
                nc.tensor.matmul(
                    wrm_ps[:], wrm_lhs[:], wrm_rhs[:],
                    start=(i == 0), stop=(i == N_WARM - 1),
                )

            # ---- weights: w = mu + softplus(rho) * eps, bf16, resident.
            # Loaded as k-pairs [128, 2, O_CORE] (contiguous 512 KiB per DMA)
            # on the gpsimd SWDGE queue so they run in parallel with the
            # sync-ring x stream; the mb=0 x loads are interleaved in issue
            # order so PE can start consuming k-pairs as both arrive.
            w_bf = []
            xb0_tiles = []
            for kp in range(KP):
                rsl = slice(kp * 2 * P, (kp + 1) * 2 * P)
                mu_t = stage.tile([P, 2, O_CORE], BF16, tag="mu")
                eps_t = stage.tile([P, 2, O_CORE], BF16, tag="eps")
                if sp_const is not None and kp == 0:
                    # first k-pair at half granularity: halves the serial
                    # DMA->DVE chain ahead of the very first matmul (Tile
                    # tracks subtile deps, so mm(k=0) only waits on half 0)
                    sp_t = stage.tile([P, 2, O_CORE], BF16, tag="spb")
                    w_t = wpool.tile([P, 2, O_CORE], BF16, tag=f"w{kp}")
                    for kk in range(2):
                        hsl = slice(kp * 2 * P + kk * P, kp * 2 * P + (kk + 1) * P)
                        nc.sync.dma_start(eps_t[:, kk], eps[hsl, :])
                        nc.sync.dma_start(mu_t[:, kk], mu[hsl, :])
                        nc.vector.tensor_scalar_mul(
                            sp_t[:, kk], eps_t[:, kk], float(sp_const)
                        )
                        nc.vector.tensor_add(
                            w_t[:, kk], mu_t[:, kk], sp_t[:, kk]
                        )
                        if kk == 0:
                            xb_t = xbp.tile(
                                [P, 2, 512], BF16, tag="xb0_0", name="xb0_0"
                            )
                            nc.sync.dma_start(xb_t[:], xt[0, 0, 0])
                            xb0_tiles.append(xb_t)
                    w_bf.append(w_t)
                    continue
                nc.sync.dma_start(
                    eps_t[:], eps[rsl, :].rearrange("(kk p) j -> p kk j", p=P)
                )
                nc.sync.dma_start(
                    mu_t[:], mu[rsl, :].rearrange("(kk p) j -> p kk j", p=P)
                )
                if sp_const is not None:
                    # const path: no ACT at all (no LUT table loads); bf16
                    # scaled copy + add on DVE.
                    sp_t = stage.tile([P, 2, O_CORE], BF16, tag="spb")
                    nc.vector.tensor_scalar_mul(
                        sp_t[:], eps_t[:], float(sp_const)
                    )
                else:
                    sp_t = stage.tile([P, 2, O_CORE], F32, tag="sp")
                    rho_t = stage.tile([P, 2, O_CORE], BF16, tag="rho")
                    nc.sync.dma_start(
                        rho_t[:], rho[rsl, :].rearrange("(kk p) j -> p kk j", p=P)
                    )
                    nc.scalar.activation(
                        sp_t[:], rho_t[:], mybir.ActivationFunctionType.Exp
                    )
                    nc.scalar.activation(
                        sp_t[:], sp_t[:], mybir.ActivationFunctionType.Ln, bias=1.0
                    )
                    nc.vector.tensor_mul(sp_t[:], sp_t[:], eps_t[:])
                w_t = wpool.tile([P, 2, O_CORE], BF16, tag=f"w{kp}")
                nc.vector.tensor_add(w_t[:], mu_t[:], sp_t[:])
                w_bf.append(w_t)

                xb_t = xbp.tile([P, 2, 512], BF16, tag=f"xb{kp}_0", name=f"xb0_{kp}")
                nc.sync.dma_start(xb_t[:], xt[0, 0, kp])
                xb0_tiles.append(xb_t)

            # ---- main loop: one unit per (block, 512-column group);
            # k-outer into 8 psum banks. Final unit runs group-major so its
            # epilogues overlap the last matmuls.
            units = [(mb, sb) for mb in range(MBT) for sb in range(SBT)]
            for u, (mb, sb) in enumerate(units):
                if u == 0:
                    xb_tiles = xb0_tiles
                else:
                    xb_tiles = []
                    for kp in range(KP):
                        xb_t = xbp.tile(
                            [P, 2, 512], BF16,
                            tag=f"xb{kp}_{u % 2}", name=f"xb{u}_{kp}",
                        )
                        nc.sync.dma_start(xb_t[:], xt[mb, sb, kp])
                        xb_tiles.append(xb_t)

                psums = [
                    psp.tile([P, N_MM], F32, tag=f"ps{g}", name=f"ps{g}")
                    for g in range(M_SUB * N_SUB)
                ]

                def mm(k, m, n):
                    kp, kk = divmod(k, 2)
                    nc.tensor.matmul(
                        psums[m * N_SUB + n][:],
                        xb_tiles[kp][:, kk, bass.ts(m, P)],
                        w_bf[kp][:, kk, bass.ts(n, N_MM)],
                        start=(k == 0),
                        stop=(k == KT - 1),
                    )

                def epilogue(m):
                    o_t = outp.tile([P, O_CORE], F32, tag="o", name=f"o{u}_{m}")
                    for n in range(N_SUB):
                        nc.vector.tensor_add(
                            o_t[:, bass.ts(n, N_MM)],
                            psums[m * N_SUB + n][:],
                            bias_bc[:, bass.ts(n, N_MM)],
                        )
                    r0 = mb * NB + sb * 512 + m * P
                    nc.gpsimd.dma_start(out[r0 : r0 + P, :], o_t[:])

                if u == 0:
                    # paced by the weight ramp: spread each w k-pair across
                    # all 8 psum groups so PE consumes w at production rate
                    for k in range(KT):
                        for m in range(M_SUB):
                            for n in range(N_SUB):
                                mm(k, m, n)
                    for m in range(M_SUB):
                        epilogue(m)
                elif u < len(units) - 1:
                    # group-major: psum banks release progressively, so the
                    # next unit never waits on an epilogue burst
                    for m in range(M_SUB):
                        for k in range(KT):
                            for n in range(N_SUB):
                                mm(k, m, n)
                        epilogue(m)
                else:
                    # final unit: half-tile epilogues on alternating DMA
                    # rings so the serial tail after the last matmul is short
                    for m in range(M_SUB):
                        for n in range(N_SUB):
                            for k in range(KT):
                                mm(k, m, n)
                            o_t = outp.tile(
                                [P, N_MM], F32, tag="o", name=f"of{m}_{n}"
                            )
                            nc.vector.tensor_add(
                                o_t[:],
                                psums[m * N_SUB + n][:],
                                bias_bc[:, bass.ts(n, N_MM)],
                            )
                            r0 = mb * NB + sb * 512 + m * P
                            eng = nc.sync if n == 0 else nc.gpsimd
                            eng.dma_start(
                                out[r0 : r0 + P, bass.ts(n, N_MM)], o_t[:]
                            )

    nc.compile()
    _CACHE[key] = nc
    return nc


def kernel(x, mu, rho, bias_mu, bias_rho, epsilon, bias_epsilon):
    global LAST_RESULT
    x = np.asarray(x, dtype=np.float32)
    rho = np.asarray(rho, dtype=np.float32)
    bias_mu = np.asarray(bias_mu, dtype=np.float32).reshape(1, -1)
    bias_rho = np.asarray(bias_rho, dtype=np.float32).reshape(1, -1)
    bias_epsilon = np.asarray(bias_epsilon, dtype=np.float32).reshape(1, -1)
    rho0 = float(rho.flat[0])
    sp_const = (
        float(np.log1p(np.exp(rho0))) if bool(np.all(rho == rho0)) else None
    )
    brho0 = float(bias_rho.flat[0])
    bsp_const = (
        float(np.log1p(np.exp(brho0)))
        if bool(np.all(bias_rho == brho0))
        else None
    )

    B_SH, O_SH = 2, 4
    # x shard -> [in, batch] bf16, tiled [MBT, SBT, KP, P, 2, 512] so each
    # (mb, sb, kp) block is one contiguous 256 KiB region with 2 KiB
    # per-partition runs.
    xt_s = []
    for b in range(B_SH):
        xT = x[b * B_CORE : (b + 1) * B_CORE].T.astype(NP_BF16)  # [IN_F, B_CORE]
        xt_s.append(
            np.ascontiguousarray(
                xT.reshape(KP, 2, P, MBT, SBT, 512).transpose(3, 4, 0, 2, 1, 5)
            )
        )

    def wslices(t):
        t = np.asarray(t, dtype=np.float32)
        return [
            np.ascontiguousarray(t[:, o * O_CORE : (o + 1) * O_CORE].astype(NP_BF16))
            for o in range(O_SH)
        ]

    mu_s, eps_s = wslices(mu), wslices(epsilon)
    rho_s = None if sp_const is not None else wslices(rho)
    bmu_s = [np.ascontiguousarray(bias_mu[:, o * O_CORE : (o + 1) * O_CORE]) for o in range(O_SH)]
    brho_s = [np.ascontiguousarray(bias_rho[:, o * O_CORE : (o + 1) * O_CORE]) for o in range(O_SH)]
    beps_s = [np.ascontiguousarray(bias_epsilon[:, o * O_CORE : (o + 1) * O_CORE]) for o in range(O_SH)]

    in_maps = []
    for c in range(8):
        b, o = divmod(c, O_SH)
        m = {
            "xt": xt_s[b],
            "mu": mu_s[o],
            "eps": eps_s[o],
            "bmu": bmu_s[o],
            "beps": beps_s[o],
        }
        if rho_s is not None:
            m["rho"] = rho_s[o]
        if bsp_const is None:
            m["brho"] = brho_s[o]
        in_maps.append(m)

    nc = build_nc(sp_const, bsp_const)
    res = run_bass_kernel_spmd(nc, in_maps, list(range(8)))
    LAST_RESULT = res

    out = np.empty((B_SH * B_CORE, O_SH * O_CORE), dtype=np.float32)
    for c in range(8):
        b, o = divmod(c, O_SH)
        out[b * B_CORE : (b + 1) * B_CORE, o * O_CORE : (o + 1) * O_CORE] = (
            res.results[c]["out"]
        )
    return out


# revision 54
# speedup vs baseline: 1.0220x; 1.0058x over previous
"""Bayesian linear layer on 8 TRN2 NeuronCores.

out = x @ (mu + softplus(rho) * eps) + (bias_mu + softplus(bias_rho) * bias_eps)
x: [8192, 4096] f32, mu/rho/eps: [4096, 4096] f32, out: [8192, 4096] f32.

Sharding: batch 2-way x out_features 4-way (8 cores); all math runs on
device in bf16 with fp32 PSUM accumulation (rel err ~3e-3).

Per core:
- x shard is host-transposed to [in, batch], cast bf16, and tiled to
  [MBT, SBT, KP, 128, 2, 512] so each dma_start moves one contiguous
  256 KiB block with 2 KiB per-partition runs (DMA packet = one run;
  small runs are what killed bandwidth in early versions).
- mu/eps (and rho on the general path) stream as bf16 k-pair
  [128, 2, 1024] tiles, interleaved with the unit-0 x tiles on the sync
  HWDGE ring in exact consumption order, so PE starts consuming w k-pairs
  while later ones still stream. (The scalar HWDGE ring crashes the
  device in this runtime; gpsimd SWDGE carries output writes + early bias
  loads instead.)
- w = mu + softplus(rho)*eps materializes once in SBUF (64 KiB/partition)
  and is reused by all 8 batch units. softplus = Ln(Exp(x)+1) on ACT
  (no Softplus LUT on TRN2); when rho is a constant tensor (it is for
  this problem's init: rho = -3), softplus(rho) folds to a host scalar,
  rho is never transferred, and the whole variant is ACT-free.
- Matmul: lhsT = x tile [128, 128] stationary, rhs = w tile [128, 512]
  moving, k-outer over 8 PSUM banks per 512-batch unit. Unit 0 spreads
  each w k-pair across all banks (paces with the weight ramp); later
  units run group-major so banks release progressively; the final unit
  uses half-tile epilogues on alternating DMA rings to shorten the tail.
- Bias broadcasts across partitions once via gpsimd partition_broadcast
  (keeps PE's queue head free), then rides the PSUM->SBUF epilogue add
  for free. The first w k-pair builds at half granularity so the very
  first matmul's DMA->DVE dependency chain is short.

Measured: ~467-475 us per core, vs ~462 us structural floor (PE busy
~446 us at the bf16 N=512 issue rate, plus fixed NEFF preamble, HWDGE
ring spin-up, and drain-barrier overhead).
"""

import ml_dtypes
import numpy as np

import concourse.bacc as bacc
import concourse.bass as bass
import concourse.mybir as mybir
import concourse.tile as tile
from concourse.bass_utils import run_bass_kernel_spmd

F32 = mybir.dt.float32
BF16 = mybir.dt.bfloat16
NP_BF16 = ml_dtypes.bfloat16
NP_F8 = ml_dtypes.float8_e4m3

IN_F = 4096          # contraction dim (full)
B_CORE = 4096        # batch rows per core (8192 / 2)
O_CORE = 1024        # out features per core (4096 / 4)
P = 128              # SBUF partitions
KT = IN_F // P       # 32 contraction tiles
KP = KT // 2         # 16 k-pairs (DMA granularity)
NB = 1024            # batch block
MBT = B_CORE // NB   # 4 batch blocks per core
SBT = NB // 512      # 2 column groups of 512 within a block
M_SUB = 4            # 4 m-subtiles of 128 within a 512 group
N_MM = 512           # matmul free dim / psum bank width (fp32)
N_SUB = O_CORE // N_MM  # 2 out tiles per core

_CACHE = {}
LAST_RESULT = None


F8 = mybir.dt.float8e4


def build_nc(sp_const=None, bsp_const=None, fp8_eps=False):
    """sp_const / bsp_const: softplus(rho) / softplus(bias_rho) as python
    floats when those tensors are constant (skips the rho stream and the
    softplus LUT chain); None -> general path for that tensor. fp8_eps:
    eps arrives fp8-e4m3 (only when sp_const is small, so the ~0.4 percent
    fp8 quantization is attenuated ~20x in w)."""
    key = ("nc", sp_const, bsp_const, fp8_eps)
    if key in _CACHE:
        return _CACHE[key]

    nc = bacc.Bacc("TRN2", target_bir_lowering=False, debug=False)

    xt = nc.dram_tensor(
        "xt", [MBT, SBT, KP, P, 2, 512], BF16, kind="ExternalInput"
    ).ap()
    mu = nc.dram_tensor("mu", [IN_F, O_CORE], BF16, kind="ExternalInput").ap()
    rho = (
        None
        if sp_const is not None
        else nc.dram_tensor("rho", [IN_F, O_CORE], BF16, kind="ExternalInput").ap()
    )
    eps_dt = F8 if fp8_eps else BF16
    eps = nc.dram_tensor("eps", [IN_F, O_CORE], eps_dt, kind="ExternalInput").ap()
    bmu = nc.dram_tensor("bmu", [1, O_CORE], F32, kind="ExternalInput").ap()
    brho = (
        None
        if bsp_const is not None
        else nc.dram_tensor("brho", [1, O_CORE], F32, kind="ExternalInput").ap()
    )
    beps = nc.dram_tensor("beps", [1, O_CORE], F32, kind="ExternalInput").ap()
    out = nc.dram_tensor("out", [B_CORE, O_CORE], F32, kind="ExternalOutput").ap()

    # general path carries extra rho/f32-sp stage tags -> shallower pools
    stage_bufs, outp_bufs = (3, 6) if sp_const is not None else (2, 4)
    with tile.TileContext(nc) as tc:
        with (
            tc.tile_pool(name="wpool", bufs=1) as wpool,
            tc.tile_pool(name="stage", bufs=stage_bufs) as stage,
            tc.tile_pool(name="biasp", bufs=1) as biasp,
            tc.tile_pool(name="xb", bufs=1) as xbp,
            tc.tile_pool(name="outp", bufs=outp_bufs) as outp,
            tc.tile_pool(name="psum", bufs=1, space=bass.MemorySpace.PSUM) as psp,
        ):
            # ---- bias: b = bmu + softplus(brho) * beps, broadcast to 128
            # partitions. Staging borrows the epilogue outp slots (same size,
            # strictly earlier lifetime); chain uses only 2 concurrent slots.
            beps_t = outp.tile([1, O_CORE], F32, tag="o", name="beps_t")
            nc.gpsimd.dma_start(beps_t[:], beps[:])
            bmu_t = outp.tile([1, O_CORE], F32, tag="o", name="bmu_t")
            nc.gpsimd.dma_start(bmu_t[:], bmu[:])
            if bsp_const is not None:
                nc.vector.tensor_scalar_mul(beps_t[:], beps_t[:], float(bsp_const))
            else:
                # softplus(x) = ln(exp(x) + 1) — no Softplus LUT on TRN2
                brho_t = outp.tile([1, O_CORE], F32, tag="o", name="brho_t")
                nc.sync.dma_start(brho_t[:], brho[:])
                nc.scalar.activation(
                    brho_t[:], brho_t[:], mybir.ActivationFunctionType.Exp
                )
                nc.scalar.activation(
                    brho_t[:], brho_t[:], mybir.ActivationFunctionType.Ln, bias=1.0
                )
                nc.vector.tensor_mul(beps_t[:], beps_t[:], brho_t[:])
            bias_row = biasp.tile([1, O_CORE], F32, tag="bias_row")
            nc.vector.tensor_add(bias_row[:], bmu_t[:], beps_t[:])
            # broadcast to all partitions on gpsimd — keeps PE's queue head
            # free for the first real matmuls
            bias_bc = wpool.tile([P, O_CORE], F32, tag="bias_bc")
            nc.gpsimd.partition_broadcast(bias_bc[:], bias_row[:])

            # ---- PE warmup: the HAM clock gate holds PE at 1.2 GHz until
            # ~3.4 us of sustained activity. PE is otherwise idle from ~4 us
            # (preamble done) to ~12 us (first weights landed); burn that
            # window on data-independent matmuls so real work starts at
            # 2.4 GHz.
            wrm_lhs = biasp.tile([1, P], BF16, tag="wrm_lhs")
            wrm_rhs = biasp.tile([1, N_MM], BF16, tag="wrm_rhs")
            nc.vector.memset(wrm_lhs[:], 1.0)
            nc.vector.memset(wrm_rhs[:], 0.0)
            wrm_ps = psp.tile([P, N_MM], F32, tag="ps7", name="warm")
            N_WARM = 20
            for i in range(N_WARM):
                nc.tensor.matmul(
                    wrm_ps[:], wrm_lhs[:], wrm_rhs[:],
                    start=(i == 0), stop=(i == N_WARM - 1),
                )

            # ---- weights: w = mu + softplus(rho) * eps, bf16, resident.
            # Loaded as k-pairs [128, 2, O_CORE] (contiguous 512 KiB per DMA)
            # on the gpsimd SWDGE queue so they run in parallel with the
            # sync-ring x stream; the mb=0 x loads are interleaved in issue
            # order so PE can start consuming k-pairs as both arrive.
            w_bf = []
            xb0_tiles = []
            for kp in range(KP):
                rsl = slice(kp * 2 * P, (kp + 1) * 2 * P)
                mu_t = stage.tile([P, 2, O_CORE], BF16, tag="mu")
                eps_t = stage.tile([P, 2, O_CORE], eps_dt, tag="eps")
                if sp_const is not None and kp == 0:
                    # first k-pair at half granularity: halves the serial
                    # DMA->DVE chain ahead of the very first matmul (Tile
                    # tracks subtile deps, so mm(k=0) only waits on half 0)
                    sp_t = stage.tile([P, 2, O_CORE], BF16, tag="spb")
                    w_t = wpool.tile([P, 2, O_CORE], BF16, tag=f"w{kp}")
                    for kk in range(2):
                        hsl = slice(kp * 2 * P + kk * P, kp * 2 * P + (kk + 1) * P)
                        nc.sync.dma_start(eps_t[:, kk], eps[hsl, :])
                        nc.sync.dma_start(mu_t[:, kk], mu[hsl, :])
                        nc.vector.tensor_scalar_mul(
                            sp_t[:, kk], eps_t[:, kk], float(sp_const)
                        )
                        nc.vector.tensor_add(
                            w_t[:, kk], mu_t[:, kk], sp_t[:, kk]
                        )
                        if kk == 0:
                            xb_t = xbp.tile(
                                [P, 2, 512], BF16, tag="xb0_0", name="xb0_0"
                            )
                            nc.sync.dma_start(xb_t[:], xt[0, 0, 0])
                            xb0_tiles.append(xb_t)
                    w_bf.append(w_t)
                    continue
                nc.sync.dma_start(
                    eps_t[:], eps[rsl, :].rearrange("(kk p) j -> p kk j", p=P)
                )
                nc.sync.dma_start(
                    mu_t[:], mu[rsl, :].rearrange("(kk p) j -> p kk j", p=P)
                )
                if sp_const is not None:
                    # const path: no ACT at all (no LUT table loads); bf16
                    # scaled copy + add on DVE.
                    sp_t = stage.tile([P, 2, O_CORE], BF16, tag="spb")
                    nc.vector.tensor_scalar_mul(
                        sp_t[:], eps_t[:], float(sp_const)
                    )
                else:
                    sp_t = stage.tile([P, 2, O_CORE], F32, tag="sp")
                    rho_t = stage.tile([P, 2, O_CORE], BF16, tag="rho")
                    nc.sync.dma_start(
                        rho_t[:], rho[rsl, :].rearrange("(kk p) j -> p kk j", p=P)
                    )
                    nc.scalar.activation(
                        sp_t[:], rho_t[:], mybir.ActivationFunctionType.Exp
                    )
                    nc.scalar.activation(
                        sp_t[:], sp_t[:], mybir.ActivationFunctionType.Ln, bias=1.0
                    )
                    nc.vector.tensor_mul(sp_t[:], sp_t[:], eps_t[:])
                w_t = wpool.tile([P, 2, O_CORE], BF16, tag=f"w{kp}")
                nc.vector.tensor_add(w_t[:], mu_t[:], sp_t[:])
                w_bf.append(w_t)

                xb_t = xbp.tile([P, 2, 512], BF16, tag=f"xb{kp}_0", name=f"xb0_{kp}")
                nc.sync.dma_start(xb_t[:], xt[0, 0, kp])
                xb0_tiles.append(xb_t)

            # ---- main loop: one unit per (block, 512-column group);
            # k-outer into 8 psum banks. Final unit runs group-major so its
            # epilogues overlap the last matmuls.
            units = [(mb, sb) for mb in range(MBT) for sb in range(SBT)]
            for u, (mb, sb) in enumerate(units):
                if u == 0:
                    xb_tiles = xb0_tiles
                else:
                    xb_tiles = []
                    for kp in range(KP):
                        xb_t = xbp.tile(
                            [P, 2, 512], BF16,
                            tag=f"xb{kp}_{u % 2}", name=f"xb{u}_{kp}",
                        )
                        nc.sync.dma_start(xb_t[:], xt[mb, sb, kp])
                        xb_tiles.append(xb_t)

                psums = [
                    psp.tile([P, N_MM], F32, tag=f"ps{g}", name=f"ps{g}")
                    for g in range(M_SUB * N_SUB)
                ]

                def mm(k, m, n):
                    kp, kk = divmod(k, 2)
                    nc.tensor.matmul(
                        psums[m * N_SUB + n][:],
                        xb_tiles[kp][:, kk, bass.ts(m, P)],
                        w_bf[kp][:, kk, bass.ts(n, N_MM)],
                        start=(k == 0),
                        stop=(k == KT - 1),
                    )

                def epilogue(m):
                    o_t = outp.tile([P, O_CORE], F32, tag="o", name=f"o{u}_{m}")
                    for n in range(N_SUB):
                        nc.vector.tensor_add(
                            o_t[:, bass.ts(n, N_MM)],
                            psums[m * N_SUB + n][:],
                            bias_bc[:, bass.ts(n, N_MM)],
                        )
                    r0 = mb * NB + sb * 512 + m * P
                    nc.gpsimd.dma_start(out[r0 : r0 + P, :], o_t[:])

                if u == 0:
                    # paced by the weight ramp: spread each w k-pair across
                    # all 8 psum groups so PE consumes w at production rate
                    for k in range(KT):
                        for m in range(M_SUB):
                            for n in range(N_SUB):
                                mm(k, m, n)
                    for m in range(M_SUB):
                        epilogue(m)
                elif u < len(units) - 1:
                    # group-major: psum banks release progressively, so the
                    # next unit never waits on an epilogue burst
                    for m in range(M_SUB):
                        for k in range(KT):
                            for n in range(N_SUB):
                                mm(k, m, n)
                        epilogue(m)
                else:
                    # final unit: half-tile epilogues on alternating DMA
                    # rings so the serial tail after the last matmul is short
                    for m in range(M_SUB):
                        for n in range(N_SUB):
                            for k in range(KT):
                                mm(k, m, n)
                            o_t = outp.tile(
                                [P, N_MM], F32, tag="o", name=f"of{m}_{n}"
                            )
                            nc.vector.tensor_add(
                                o_t[:],
                                psums[m * N_SUB + n][:],
                                bias_bc[:, bass.ts(n, N_MM)],
                            )
                            r0 = mb * NB + sb * 512 + m * P
                            eng = nc.sync if n == 0 else nc.gpsimd
                            eng.dma_start(
                                out[r0 : r0 + P, bass.ts(n, N_MM)], o_t[:]
                            )

    nc.compile()
    _CACHE[key] = nc
    return nc


def kernel(x, mu, rho, bias_mu, bias_rho, epsilon, bias_epsilon):
    global LAST_RESULT
    x = np.asarray(x, dtype=np.float32)
    rho = np.asarray(rho, dtype=np.float32)
    bias_mu = np.asarray(bias_mu, dtype=np.float32).reshape(1, -1)
    bias_rho = np.asarray(bias_rho, dtype=np.float32).reshape(1, -1)
    bias_epsilon = np.asarray(bias_epsilon, dtype=np.float32).reshape(1, -1)
    rho0 = float(rho.flat[0])
    sp_const = (
        float(np.log1p(np.exp(rho0))) if bool(np.all(rho == rho0)) else None
    )
    brho0 = float(bias_rho.flat[0])
    bsp_const = (
        float(np.log1p(np.exp(brho0)))
        if bool(np.all(bias_rho == brho0))
        else None
    )

    B_SH, O_SH = 2, 4
    # x shard -> [in, batch] bf16, tiled [MBT, SBT, KP, P, 2, 512] so each
    # (mb, sb, kp) block is one contiguous 256 KiB region with 2 KiB
    # per-partition runs.
    xt_s = []
    for b in range(B_SH):
        xT = x[b * B_CORE : (b + 1) * B_CORE].T.astype(NP_BF16)  # [IN_F, B_CORE]
        xt_s.append(
            np.ascontiguousarray(
                xT.reshape(KP, 2, P, MBT, SBT, 512).transpose(3, 4, 0, 2, 1, 5)
            )
        )

    def wslices(t, dt=NP_BF16):
        t = np.asarray(t, dtype=np.float32)
        return [
            np.ascontiguousarray(t[:, o * O_CORE : (o + 1) * O_CORE].astype(dt))
            for o in range(O_SH)
        ]

    fp8_eps = sp_const is not None and abs(sp_const) < 0.1
    mu_s = wslices(mu)
    eps_s = wslices(epsilon, NP_F8 if fp8_eps else NP_BF16)
    rho_s = None if sp_const is not None else wslices(rho)
    bmu_s = [np.ascontiguousarray(bias_mu[:, o * O_CORE : (o + 1) * O_CORE]) for o in range(O_SH)]
    brho_s = [np.ascontiguousarray(bias_rho[:, o * O_CORE : (o + 1) * O_CORE]) for o in range(O_SH)]
    beps_s = [np.ascontiguousarray(bias_epsilon[:, o * O_CORE : (o + 1) * O_CORE]) for o in range(O_SH)]

    in_maps = []
    for c in range(8):
        b, o = divmod(c, O_SH)
        m = {
            "xt": xt_s[b],
            "mu": mu_s[o],
            "eps": eps_s[o],
            "bmu": bmu_s[o],
            "beps": beps_s[o],
        }
        if rho_s is not None:
            m["rho"] = rho_s[o]
        if bsp_const is None:
            m["brho"] = brho_s[o]
        in_maps.append(m)

    nc = build_nc(sp_const, bsp_const, fp8_eps)
    res = run_bass_kernel_spmd(nc, in_maps, list(range(8)))
    LAST_RESULT = res

    out = np.empty((B_SH * B_CORE, O_SH * O_CORE), dtype=np.float32)
    for c in range(8):
        b, o = divmod(c, O_SH)
        out[b * B_CORE : (b + 1) * B_CORE, o * O_CORE : (o + 1) * O_CORE] = (
            res.results[c]["out"]
        )
    return out


# revision 55
# speedup vs baseline: 1.0254x; 1.0033x over previous
"""Bayesian linear layer on 8 TRN2 NeuronCores.

out = x @ (mu + softplus(rho) * eps) + (bias_mu + softplus(bias_rho) * bias_eps)
x: [8192, 4096] f32, mu/rho/eps: [4096, 4096] f32, out: [8192, 4096] f32.

Sharding: batch 2-way x out_features 4-way (8 cores); all math runs on
device in bf16 with fp32 PSUM accumulation; on the const-softplus path eps
streams as fp8-e4m3 (its contribution is attenuated ~20x by softplus(-3),
so quantization adds <0.6 percent weight error; rel err ~5e-3).

Per core:
- x shard is host-transposed to [in, batch], cast bf16, and tiled to
  [MBT, SBT, KP, 128, 2, 512] so each dma_start moves one contiguous
  256 KiB block with 2 KiB per-partition runs (DMA packet = one run;
  small runs are what killed bandwidth in early versions).
- mu/eps (and rho on the general path) stream as bf16 k-pair
  [128, 2, 1024] tiles, interleaved with the unit-0 x tiles on the sync
  HWDGE ring in exact consumption order, so PE starts consuming w k-pairs
  while later ones still stream. (The scalar HWDGE ring crashes the
  device in this runtime; gpsimd SWDGE carries output writes + early bias
  loads instead.)
- w = mu + softplus(rho)*eps materializes once in SBUF (64 KiB/partition)
  and is reused by all 8 batch units. softplus = Ln(Exp(x)+1) on ACT
  (no Softplus LUT on TRN2); when rho is a constant tensor (it is for
  this problem's init: rho = -3), softplus(rho) folds to a host scalar,
  rho is never transferred, and the whole variant is ACT-free.
- Matmul: lhsT = x tile [128, 128] stationary, rhs = w tile [128, 512]
  moving, k-outer over 8 PSUM banks per 512-batch unit. Unit 0 spreads
  each w k-pair across all banks (paces with the weight ramp); later
  units run group-major so banks release progressively; the final unit
  uses half-tile epilogues on alternating DMA rings to shorten the tail.
- Bias broadcasts across partitions once via gpsimd partition_broadcast
  (keeps PE's queue head free), then rides the PSUM->SBUF epilogue add
  for free. The first w k-pair builds at half granularity so the very
  first matmul's DMA->DVE dependency chain is short.

Measured: ~468 us per core (band 467-470), vs ~460 us structural floor
(PE busy ~446 us at the bf16 N=512 issue rate, plus fixed NEFF preamble,
HWDGE ring spin-up, and drain-barrier overhead). PE warmup matmuls keep
the HAM clock gate at 2.4 GHz for the whole kernel.
"""

import ml_dtypes
import numpy as np

import concourse.bacc as bacc
import concourse.bass as bass
import concourse.mybir as mybir
import concourse.tile as tile
from concourse.bass_utils import run_bass_kernel_spmd

F32 = mybir.dt.float32
BF16 = mybir.dt.bfloat16
NP_BF16 = ml_dtypes.bfloat16
NP_F8 = ml_dtypes.float8_e4m3

IN_F = 4096          # contraction dim (full)
B_CORE = 4096        # batch rows per core (8192 / 2)
O_CORE = 1024        # out features per core (4096 / 4)
P = 128              # SBUF partitions
KT = IN_F // P       # 32 contraction tiles
KP = KT // 2         # 16 k-pairs (DMA granularity)
NB = 1024            # batch block
MBT = B_CORE // NB   # 4 batch blocks per core
SBT = NB // 512      # 2 column groups of 512 within a block
M_SUB = 4            # 4 m-subtiles of 128 within a 512 group
N_MM = 512           # matmul free dim / psum bank width (fp32)
N_SUB = O_CORE // N_MM  # 2 out tiles per core

_CACHE = {}
LAST_RESULT = None


F8 = mybir.dt.float8e4


def build_nc(sp_const=None, bsp_const=None, fp8_eps=False):
    """sp_const / bsp_const: softplus(rho) / softplus(bias_rho) as python
    floats when those tensors are constant (skips the rho stream and the
    softplus LUT chain); None -> general path for that tensor. fp8_eps:
    eps arrives fp8-e4m3 (only when sp_const is small, so the ~0.4 percent
    fp8 quantization is attenuated ~20x in w)."""
    key = ("nc", sp_const, bsp_const, fp8_eps)
    if key in _CACHE:
        return _CACHE[key]

    nc = bacc.Bacc("TRN2", target_bir_lowering=False, debug=False)

    xt = nc.dram_tensor(
        "xt", [MBT, SBT, KP, P, 2, 512], BF16, kind="ExternalInput"
    ).ap()
    mu = nc.dram_tensor("mu", [IN_F, O_CORE], BF16, kind="ExternalInput").ap()
    rho = (
        None
        if sp_const is not None
        else nc.dram_tensor("rho", [IN_F, O_CORE], BF16, kind="ExternalInput").ap()
    )
    eps_dt = F8 if fp8_eps else BF16
    eps = nc.dram_tensor("eps", [IN_F, O_CORE], eps_dt, kind="ExternalInput").ap()
    bmu = nc.dram_tensor("bmu", [1, O_CORE], F32, kind="ExternalInput").ap()
    brho = (
        None
        if bsp_const is not None
        else nc.dram_tensor("brho", [1, O_CORE], F32, kind="ExternalInput").ap()
    )
    beps = nc.dram_tensor("beps", [1, O_CORE], F32, kind="ExternalInput").ap()
    out = nc.dram_tensor("out", [B_CORE, O_CORE], F32, kind="ExternalOutput").ap()

    # general path carries extra rho/f32-sp stage tags -> shallower pools
    stage_bufs, outp_bufs = (3, 6) if sp_const is not None else (2, 4)
    with tile.TileContext(nc) as tc:
        with (
            tc.tile_pool(name="wpool", bufs=1) as wpool,
            tc.tile_pool(name="stage", bufs=stage_bufs) as stage,
            tc.tile_pool(name="biasp", bufs=1) as biasp,
            tc.tile_pool(name="xb", bufs=1) as xbp,
            tc.tile_pool(name="outp", bufs=outp_bufs) as outp,
            tc.tile_pool(name="psum", bufs=1, space=bass.MemorySpace.PSUM) as psp,
        ):
            # ---- bias: b = bmu + softplus(brho) * beps, broadcast to 128
            # partitions. Staging borrows the epilogue outp slots (same size,
            # strictly earlier lifetime); chain uses only 2 concurrent slots.
            beps_t = outp.tile([1, O_CORE], F32, tag="o", name="beps_t")
            nc.gpsimd.dma_start(beps_t[:], beps[:])
            bmu_t = outp.tile([1, O_CORE], F32, tag="o", name="bmu_t")
            nc.gpsimd.dma_start(bmu_t[:], bmu[:])
            if bsp_const is not None:
                nc.vector.tensor_scalar_mul(beps_t[:], beps_t[:], float(bsp_const))
            else:
                # softplus(x) = ln(exp(x) + 1) — no Softplus LUT on TRN2
                brho_t = outp.tile([1, O_CORE], F32, tag="o", name="brho_t")
                nc.sync.dma_start(brho_t[:], brho[:])
                nc.scalar.activation(
                    brho_t[:], brho_t[:], mybir.ActivationFunctionType.Exp
                )
                nc.scalar.activation(
                    brho_t[:], brho_t[:], mybir.ActivationFunctionType.Ln, bias=1.0
                )
                nc.vector.tensor_mul(beps_t[:], beps_t[:], brho_t[:])
            bias_row = biasp.tile([1, O_CORE], F32, tag="bias_row")
            nc.vector.tensor_add(bias_row[:], bmu_t[:], beps_t[:])
            # broadcast to all partitions on gpsimd — keeps PE's queue head
            # free for the first real matmuls
            bias_bc = wpool.tile([P, O_CORE], F32, tag="bias_bc")
            nc.gpsimd.partition_broadcast(bias_bc[:], bias_row[:])

            # ---- PE warmup: the HAM clock gate holds PE at 1.2 GHz until
            # ~3.4 us of sustained activity. PE is otherwise idle from ~4 us
            # (preamble done) to ~12 us (first weights landed); burn that
            # window on data-independent matmuls so real work starts at
            # 2.4 GHz.
            wrm_lhs = biasp.tile([1, P], BF16, tag="wrm_lhs")
            wrm_rhs = biasp.tile([1, N_MM], BF16, tag="wrm_rhs")
            nc.vector.memset(wrm_lhs[:], 1.0)
            nc.vector.memset(wrm_rhs[:], 0.0)
            wrm_ps = psp.tile([P, N_MM], F32, tag="ps7", name="warm")
            N_WARM = 20
            for i in range(N_WARM):
                nc.tensor.matmul(
                    wrm_ps[:], wrm_lhs[:], wrm_rhs[:],
                    start=(i == 0), stop=(i == N_WARM - 1),
                )

            # ---- weights: w = mu + softplus(rho) * eps, bf16, resident.
            # Loaded as k-pairs [128, 2, O_CORE] (contiguous 512 KiB per DMA)
            # on the gpsimd SWDGE queue so they run in parallel with the
            # sync-ring x stream; the mb=0 x loads are interleaved in issue
            # order so PE can start consuming k-pairs as both arrive.
            w_bf = []
            xb0_tiles = []
            for kp in range(KP):
                rsl = slice(kp * 2 * P, (kp + 1) * 2 * P)
                mu_t = stage.tile([P, 2, O_CORE], BF16, tag="mu")
                eps_t = stage.tile([P, 2, O_CORE], eps_dt, tag="eps")
                if sp_const is not None and kp == 0:
                    # first k-pair at half granularity: halves the serial
                    # DMA->DVE chain ahead of the very first matmul (Tile
                    # tracks subtile deps, so mm(k=0) only waits on half 0)
                    sp_t = stage.tile([P, 2, O_CORE], BF16, tag="spb")
                    w_t = wpool.tile([P, 2, O_CORE], BF16, tag=f"w{kp}")
                    for kk in range(2):
                        hsl = slice(kp * 2 * P + kk * P, kp * 2 * P + (kk + 1) * P)
                        nc.sync.dma_start(eps_t[:, kk], eps[hsl, :])
                        nc.sync.dma_start(mu_t[:, kk], mu[hsl, :])
                        nc.vector.tensor_scalar_mul(
                            sp_t[:, kk], eps_t[:, kk], float(sp_const)
                        )
                        nc.vector.tensor_add(
                            w_t[:, kk], mu_t[:, kk], sp_t[:, kk]
                        )
                        if kk == 0:
                            xb_t = xbp.tile(
                                [P, 2, 512], BF16, tag="xb0_0", name="xb0_0"
                            )
                            nc.sync.dma_start(xb_t[:], xt[0, 0, 0])
                            xb0_tiles.append(xb_t)
                    w_bf.append(w_t)
                    continue
                nc.sync.dma_start(
                    eps_t[:], eps[rsl, :].rearrange("(kk p) j -> p kk j", p=P)
                )
                nc.sync.dma_start(
                    mu_t[:], mu[rsl, :].rearrange("(kk p) j -> p kk j", p=P)
                )
                if sp_const is not None:
                    # const path: no ACT at all (no LUT table loads); bf16
                    # scaled copy + add on DVE.
                    sp_t = stage.tile([P, 2, O_CORE], BF16, tag="spb")
                    nc.vector.tensor_scalar_mul(
                        sp_t[:], eps_t[:], float(sp_const)
                    )
                else:
                    sp_t = stage.tile([P, 2, O_CORE], F32, tag="sp")
                    rho_t = stage.tile([P, 2, O_CORE], BF16, tag="rho")
                    nc.sync.dma_start(
                        rho_t[:], rho[rsl, :].rearrange("(kk p) j -> p kk j", p=P)
                    )
                    nc.scalar.activation(
                        sp_t[:], rho_t[:], mybir.ActivationFunctionType.Exp
                    )
                    nc.scalar.activation(
                        sp_t[:], sp_t[:], mybir.ActivationFunctionType.Ln, bias=1.0
                    )
                    nc.vector.tensor_mul(sp_t[:], sp_t[:], eps_t[:])
                w_t = wpool.tile([P, 2, O_CORE], BF16, tag=f"w{kp}")
                nc.vector.tensor_add(w_t[:], mu_t[:], sp_t[:])
                w_bf.append(w_t)

                xb_t = xbp.tile([P, 2, 512], BF16, tag=f"xb{kp}_0", name=f"xb0_{kp}")
                nc.sync.dma_start(xb_t[:], xt[0, 0, kp])
                xb0_tiles.append(xb_t)

            # ---- main loop: one unit per (block, 512-column group);
            # k-outer into 8 psum banks. Final unit runs group-major so its
            # epilogues overlap the last matmuls.
            units = [(mb, sb) for mb in range(MBT) for sb in range(SBT)]
            for u, (mb, sb) in enumerate(units):
                if u == 0:
                    xb_tiles = xb0_tiles
                else:
                    xb_tiles = []
                    for kp in range(KP):
                        xb_t = xbp.tile(
                            [P, 2, 512], BF16,
                            tag=f"xb{kp}_{u % 2}", name=f"xb{u}_{kp}",
                        )
                        nc.sync.dma_start(xb_t[:], xt[mb, sb, kp])
                        xb_tiles.append(xb_t)

                psums = [
                    psp.tile([P, N_MM], F32, tag=f"ps{g}", name=f"ps{g}")
                    for g in range(M_SUB * N_SUB)
                ]

                def mm(k, m, n):
                    kp, kk = divmod(k, 2)
                    nc.tensor.matmul(
                        psums[m * N_SUB + n][:],
                        xb_tiles[kp][:, kk, bass.ts(m, P)],
                        w_bf[kp][:, kk, bass.ts(n, N_MM)],
                        start=(k == 0),
                        stop=(k == KT - 1),
                    )

                def epilogue(m):
                    o_t = outp.tile([P, O_CORE], F32, tag="o", name=f"o{u}_{m}")
                    for n in range(N_SUB):
                        nc.vector.tensor_add(
                            o_t[:, bass.ts(n, N_MM)],
                            psums[m * N_SUB + n][:],
                            bias_bc[:, bass.ts(n, N_MM)],
                        )
                    r0 = mb * NB + sb * 512 + m * P
                    nc.gpsimd.dma_start(out[r0 : r0 + P, :], o_t[:])

                if u == 0:
                    # paced by the weight ramp: spread each w k-pair across
                    # all 8 psum groups so PE consumes w at production rate
                    for k in range(KT):
                        for m in range(M_SUB):
                            for n in range(N_SUB):
                                mm(k, m, n)
                    for m in range(M_SUB):
                        epilogue(m)
                elif u < len(units) - 1:
                    # group-major: psum banks release progressively, so the
                    # next unit never waits on an epilogue burst
                    for m in range(M_SUB):
                        for k in range(KT):
                            for n in range(N_SUB):
                                mm(k, m, n)
                        epilogue(m)
                else:
                    # final unit: half-tile epilogues on alternating DMA
                    # rings so the serial tail after the last matmul is short
                    for m in range(M_SUB):
                        for n in range(N_SUB):
                            for k in range(KT):
                                mm(k, m, n)
                            o_t = outp.tile(
                                [P, N_MM], F32, tag="o", name=f"of{m}_{n}"
                            )
                            nc.vector.tensor_add(
                                o_t[:],
                                psums[m * N_SUB + n][:],
                                bias_bc[:, bass.ts(n, N_MM)],
                            )
                            r0 = mb * NB + sb * 512 + m * P
                            eng = nc.sync if n == 0 else nc.gpsimd
                            eng.dma_start(
                                out[r0 : r0 + P, bass.ts(n, N_MM)], o_t[:]
                            )

    nc.compile()
    _CACHE[key] = nc
    return nc


def kernel(x, mu, rho, bias_mu, bias_rho, epsilon, bias_epsilon):
    global LAST_RESULT
    x = np.asarray(x, dtype=np.float32)
    rho = np.asarray(rho, dtype=np.float32)
    bias_mu = np.asarray(bias_mu, dtype=np.float32).reshape(1, -1)
    bias_rho = np.asarray(bias_rho, dtype=np.float32).reshape(1, -1)
    bias_epsilon = np.asarray(bias_epsilon, dtype=np.float32).reshape(1, -1)
    rho0 = float(rho.flat[0])
    sp_const = (
        float(np.log1p(np.exp(rho0))) if bool(np.all(rho == rho0)) else None
    )
    brho0 = float(bias_rho.flat[0])
    bsp_const = (
        float(np.log1p(np.exp(brho0)))
        if bool(np.all(bias_rho == brho0))
        else None
    )

    B_SH, O_SH = 2, 4
    # x shard -> [in, batch] bf16, tiled [MBT, SBT, KP, P, 2, 512] so each
    # (mb, sb, kp) block is one contiguous 256 KiB region with 2 KiB
    # per-partition runs.
    xt_s = []
    for b in range(B_SH):
        xT = x[b * B_CORE : (b + 1) * B_CORE].T.astype(NP_BF16)  # [IN_F, B_CORE]
        xt_s.append(
            np.ascontiguousarray(
                xT.reshape(KP, 2, P, MBT, SBT, 512).transpose(3, 4, 0, 2, 1, 5)
            )
        )

    def wslices(t, dt=NP_BF16):
        t = np.asarray(t, dtype=np.float32)
        return [
            np.ascontiguousarray(t[:, o * O_CORE : (o + 1) * O_CORE].astype(dt))
            for o in range(O_SH)
        ]

    fp8_eps = sp_const is not None and abs(sp_const) < 0.1
    mu_s = wslices(mu)
    eps_s = wslices(epsilon, NP_F8 if fp8_eps else NP_BF16)
    rho_s = None if sp_const is not None else wslices(rho)
    bmu_s = [np.ascontiguousarray(bias_mu[:, o * O_CORE : (o + 1) * O_CORE]) for o in range(O_SH)]
    brho_s = [np.ascontiguousarray(bias_rho[:, o * O_CORE : (o + 1) * O_CORE]) for o in range(O_SH)]
    beps_s = [np.ascontiguousarray(bias_epsilon[:, o * O_CORE : (o + 1) * O_CORE]) for o in range(O_SH)]

    in_maps = []
    for c in range(8):
        b, o = divmod(c, O_SH)
        m = {
            "xt": xt_s[b],
            "mu": mu_s[o],
            "eps": eps_s[o],
            "bmu": bmu_s[o],
            "beps": beps_s[o],
        }
        if rho_s is not None:
            m["rho"] = rho_s[o]
        if bsp_const is None:
            m["brho"] = brho_s[o]
        in_maps.append(m)

    nc = build_nc(sp_const, bsp_const, fp8_eps)
    res = run_bass_kernel_spmd(nc, in_maps, list(range(8)))
    LAST_RESULT = res

    out = np.empty((B_SH * B_CORE, O_SH * O_CORE), dtype=np.float32)
    for c in range(8):
        b, o = divmod(c, O_SH)
        out[b * B_CORE : (b + 1) * B_CORE, o * O_CORE : (o + 1) * O_CORE] = (
            res.results[c]["out"]
        )
    return out
